# revision 13
# baseline (speedup 1.0000x reference)
"""Grouped-Query Attention (B=2, T=2048, H=2048, 16 q-heads, 4 kv-heads, d=128,
causal) on 8 Trainium2 NeuronCores.

Sharding: core c = (batch b, kv-group g) with b = c // 4, g = c % 4.
Each core handles one batch element, one kv head, and its 4 q heads:
  - Q/K/V projections for its slice (tensor-parallel over heads)
  - causal attention for 4 q heads against the shared K/V head
  - partial o_proj (row-parallel): out_partial = O_heads @ w_o[:, cols].T
Host sums the 4 per-batch partials (the row-parallel all-reduce) and stacks.

Device layouts (chosen so no transposes are ever needed on-chip):
  QT, KT: [d=128, T]  (projection computed directly transposed)
  V:      [T-tile=128, d]
  scores: computed directly transposed as ST [k, q] via lhsT=KT_j, rhs=QT
  P = exp(ST/sqrt(d)) stays [k, q] and feeds PV as rhs -> OT [d, q] which is
  exactly the lhsT the o_proj needs. Row sums of P (softmax denominator) are
  computed broadcast via an all-ones [128,128] stationary matmul.
All matmul inputs bf16, PSUM accumulation fp32, softmax in fp32.

Perf structure:
  - every input tensor is repacked on the host so its DMA is a single
    transfer (or a few) with long contiguous per-partition rows; the DMA
    queue costs ~625ns per transfer to issue, so many small transfers
    starve the PE.
  - PE warm-up matmuls at t=0 release the HAM clock gate early.
  - score tiles st are single-bank [128,512] PSUM tiles with bufs=4 so the
    PE can run several tiles ahead of the scalar-engine exp.
  - o_proj for chunk qc is emitted interleaved into the attention head loop
    of chunk qc+1 (software pipelining) so PSUM->SBUF casts and stage DMAs
    never serialize against the PE.
  - softmax: denominator reciprocal via DVE reciprocal_approx_fast (~51 ULP,
    5x faster than exact); the OT*(1/l) normalize runs on gpsimd, as does one
    of the four diagonal mask multiplies, to keep DVE under the PE's rate.
  - o_proj PSUM->bf16 casts alternate between scalar and vector engines.
"""

import numpy as np
import ml_dtypes
from contextlib import ExitStack

import concourse.bass as bass
import concourse.mybir as mybir
import concourse.tile as tile
from concourse.bass_utils import run_bass_kernel_spmd

# ---------------------------------------------------------------------------
# Workaround for this compiler build's per-instruction sync-wait-slot limit
# (walrus setupSyncWait rejects >2 waits on an instruction). Post-process the
# serialized BIR: any instruction carrying more than 2 sem waits gets the
# excess moved onto injected same-engine Drain instructions placed directly
# before it (same queue, program order => identical semantics).
import json as _json

_WAIT_LIMITS = {}
_WAIT_LIMIT_DEFAULT = 1
_orig_to_json_bytes = bass.Bass.to_json_bytes


def _split_waits_json(bj: bytes) -> bytes:
    m = _json.loads(bj)
    ctr = 0
    changed = False
    for f in m["functions"]:
        for blk in f["blocks"]:
            out = []
            for inst in blk["instructions"]:
                si = inst.get("sync_info") or {}
                w = si.get("on_wait") or []
                lim = _WAIT_LIMITS.get(inst.get("opcode"), _WAIT_LIMIT_DEFAULT)
                if len(w) > lim:
                    changed = True
                    extra, keep = w[:-lim], w[-lim:]
                    si["on_wait"] = keep
                    for i in range(0, len(extra), 1):
                        ctr += 1
                        out.append({
                            "debug": inst.get("debug", 0),
                            "engine": inst["engine"],
                            "ins": [],
                            "is_reset_sema": False,
                            "name": f"I-wsplit-{ctr}",
                            "opcode": "Drain",
                            "outs": [],
                            "sync_info": {
                                "on_update": [],
                                "on_wait": extra[i:i + 1],
                            },
                        })
                out.append(inst)
            if changed:
                blk["instructions"] = out
    if not changed:
        return bj
    return _json.dumps(m).encode()


def _to_json_bytes_patched(self, *a, **k):
    return _split_waits_json(_orig_to_json_bytes(self, *a, **k))


bass.Bass.to_json_bytes = _to_json_bytes_patched
# ---------------------------------------------------------------------------

HIDDEN = 2048
N_HEADS = 16
N_KV = 4
HD = 128
B, T = 2, 2048
G = N_HEADS // N_KV          # q heads per core = 4
HC = HIDDEN // 128           # contraction chunks = 16
NCORES = 8
SCALE = HD ** -0.5

BF16 = mybir.dt.bfloat16
F32 = mybir.dt.float32

_CACHE = {}
LAST_RESULTS = None


def _build_program():
    nc = bass.Bass("TRN2")
    # host-repacked inputs: partition dim first, long contiguous rows
    xb = nc.dram_tensor("xb", [128, 4, HC, 512], BF16, kind="ExternalInput")
    wq = nc.dram_tensor("wq", [128, HC, G * HD], BF16, kind="ExternalInput")
    wk = nc.dram_tensor("wk", [128, HC, HD], BF16, kind="ExternalInput")
    wv = nc.dram_tensor("wv", [128, HC, HD], BF16, kind="ExternalInput")
    wo = nc.dram_tensor("wo", [128, G, HIDDEN], BF16, kind="ExternalInput")
    msk = nc.dram_tensor("msk", [128, 2, 1024], BF16, kind="ExternalInput")
    out = nc.dram_tensor("out", [T, HIDDEN], BF16, kind="ExternalOutput")

    EXP = mybir.ActivationFunctionType.Exp

    with tile.TileContext(nc) as tc, ExitStack() as ctx:
        sing = ctx.enter_context(tc.tile_pool(name="sing", bufs=1))
        ptp = ctx.enter_context(tc.tile_pool(name="ptp", bufs=16))
        vecp = ctx.enter_context(tc.tile_pool(name="vecp", bufs=3))
        otnp = ctx.enter_context(tc.tile_pool(name="otnp", bufs=8))
        outp = ctx.enter_context(tc.tile_pool(name="outp", bufs=3))
        psum = ctx.enter_context(tc.tile_pool(name="psum", bufs=2, space="PSUM"))

        xT_sb = sing.tile([128, 4, HC, 512], BF16)
        wq_sb = sing.tile([128, HC, G * HD], BF16)
        wk_sb = sing.tile([128, HC, HD], BF16)
        wv_sb = sing.tile([128, HC, HD], BF16)
        wo_sb = sing.tile([128, G, HIDDEN], BF16)
        msk_sb = sing.tile([128, 2, 1024], BF16)
        ones_sb = sing.tile([128, 128], BF16)
        warm_sb = sing.tile([128, 512], BF16)
        qt_sb = sing.tile([128, G, T], BF16)
        kt_sb = sing.tile([128, T], BF16)
        vt_sb = sing.tile([128, T], BF16)
        v_sb = sing.tile([128, HC, HD], BF16)

        nc.vector.memset(ones_sb, 1.0)
        nc.vector.memset(warm_sb, 0.0)

        # --- PE warm-up: release the HAM clock gate before real work ---
        for w in range(16):
            wp = psum.tile([128, 1024], F32, tag="big", bufs=2, name=f"warm_{w}")
            nc.tensor.matmul(wp[:, 0:512], lhsT=ones_sb, rhs=warm_sb,
                             start=True, stop=True)

        # --- input DMAs: few big transfers, K/V/x-block0 first ---
        nc.sync.dma_start(out=wk_sb, in_=wk[:, :, :])
        for qtr in range(4):
            nc.sync.dma_start(out=xT_sb[:, 0, 4 * qtr:4 * qtr + 4],
                              in_=xb[:, 0, 4 * qtr:4 * qtr + 4])
        nc.sync.dma_start(out=wv_sb, in_=wv[:, :, :])
        nc.sync.dma_start(out=xT_sb[:, 1], in_=xb[:, 1])
        nc.sync.dma_start(out=wq_sb, in_=wq[:, :, :])
        for t4 in range(2, 4):
            nc.sync.dma_start(out=xT_sb[:, t4], in_=xb[:, t4])
        nc.sync.dma_start(out=msk_sb, in_=msk[:, :, :])
        nc.sync.dma_start(out=wo_sb, in_=wo[:, :, :])

        # ---- projections (per T-chunk) ----
        for t4 in range(4):
            tsl = slice(t4 * 512, (t4 + 1) * 512)
            # K projection -> KT [d, 512]
            kp = psum.tile([128, 512], F32, tag="ls", bufs=2, name=f"kp_{t4}")
            for c in range(HC):
                nc.tensor.matmul(
                    kp, lhsT=wk_sb[:, c, :], rhs=xT_sb[:, t4, c, :],
                    start=(c == 0), stop=(c == HC - 1),
                )
            nc.scalar.copy(kt_sb[:, tsl], kp)
            # V projection -> VT [d, 512], then XBAR-transpose to V [t, d]
            vtp = psum.tile([128, 512], F32, tag="ot", bufs=2, name=f"vtp_{t4}")
            for c in range(HC):
                nc.tensor.matmul(
                    vtp, lhsT=wv_sb[:, c, :], rhs=xT_sb[:, t4, c, :],
                    start=(c == 0), stop=(c == HC - 1),
                )
            nc.scalar.copy(vt_sb[:, tsl], vtp)
            for ts in range(4):
                tt = 4 * t4 + ts
                nc.sync.dma_start_transpose(
                    out=v_sb[:, tt, :], in_=vt_sb[:, tt * 128:(tt + 1) * 128])
            # Q projection, head pairs -> QT [d, h, 512]
            for hp in range(2):
                qp = psum.tile([128, 1024], F32, tag="big", bufs=2,
                               name=f"qp_{t4}_{hp}")
                for half in range(2):
                    h = 2 * hp + half
                    for c in range(HC):
                        nc.tensor.matmul(
                            qp[:, half * 512:(half + 1) * 512],
                            lhsT=wq_sb[:, c, h * HD:(h + 1) * HD],
                            rhs=xT_sb[:, t4, c, :],
                            start=(c == 0), stop=(c == HC - 1),
                        )
                nc.scalar.copy(qt_sb[:, 2 * hp:2 * hp + 2, tsl], qp)

        # ---- attention with o_proj software-pipelined one chunk behind ----
        otns = {}

        def emit_oproj(qc, tt):
            stage = outp.tile([128, HIDDEN], BF16, tag="stage", bufs=3,
                              name=f"stage_{qc}_{tt}")
            for ecp in range(2):
                op = psum.tile([128, 1024], F32, tag="big", bufs=2,
                               name=f"op_{qc}_{tt}_{ecp}")
                for half in range(2):
                    ec = 2 * ecp + half
                    for h in range(G):
                        nc.tensor.matmul(
                            op[:, half * 512:(half + 1) * 512],
                            lhsT=otns[(qc, h)][:, tt * 128:(tt + 1) * 128],
                            rhs=wo_sb[:, h, ec * 512:(ec + 1) * 512],
                            start=(h == 0), stop=(h == G - 1),
                        )
                esl = slice(ecp * 1024, (ecp + 1) * 1024)
                nc.vector.tensor_copy(stage[:, esl], op)
            r0 = qc * 512 + tt * 128
            nc.sync.dma_start(out=out[r0:r0 + 128, :], in_=stage)

        for qc in range(4):
            qsl = slice(qc * 512, (qc + 1) * 512)
            njt = 4 * qc + 4
            npair = njt // 2
            # pair index list, diagonal pairs first so their mask ops don't
            # pile up late
            ps = list(range(2 * qc, npair)) + list(range(0, 2 * qc))
            for h in range(G):
                ot = psum.tile([128, 512], F32, tag="ot", bufs=2,
                               name=f"ot_{qc}_{h}")
                ls = psum.tile([128, 512], F32, tag="ls", bufs=2,
                               name=f"ls_{qc}_{h}")
                # software pipeline: PV/LS lag the S matmuls by several
                # pairs so the exp (+ mask) latency never stalls the
                # in-order PE queue.
                depth = min(npair - 1, 3)
                pend = []
                n_emitted = 0

                def flush_one():
                    nonlocal n_emitted
                    pp_idx, pp = pend.pop(0)
                    for half in range(2):
                        j = 2 * pp_idx + half
                        nc.tensor.matmul(ot, lhsT=v_sb[:, j, :],
                                         rhs=pp[:, half * 512:(half + 1) * 512],
                                         start=(n_emitted == 0 and half == 0),
                                         stop=(n_emitted == npair - 1 and half == 1))
                        nc.tensor.matmul(ls, lhsT=ones_sb,
                                         rhs=pp[:, half * 512:(half + 1) * 512],
                                         start=(n_emitted == 0 and half == 0),
                                         stop=(n_emitted == npair - 1 and half == 1))
                    n_emitted += 1

                for p in ps:
                    stp = psum.tile([128, 1024], F32, tag="big", bufs=2,
                                    name=f"stp_{qc}_{h}_{p}")
                    for half in range(2):
                        j = 2 * p + half
                        nc.tensor.matmul(
                            stp[:, half * 512:(half + 1) * 512],
                            lhsT=kt_sb[:, j * 128:(j + 1) * 128],
                            rhs=qt_sb[:, h, qsl], start=True, stop=True,
                        )
                    pt = ptp.tile([128, 1024], BF16, tag="pt", bufs=8,
                                  name=f"pt_{qc}_{h}_{p}")
                    nc.scalar.activation(pt, stp, EXP, scale=float(SCALE))
                    if p >= 2 * qc:
                        nc.vector.tensor_mul(pt, pt, msk_sb[:, p - 2 * qc, :])
                    pend.append((p, pt))
                    if len(pend) > depth:
                        flush_one()
                while pend:
                    flush_one()
                lnl = vecp.tile([128, 512], F32, tag="lnl", bufs=3,
                                name=f"lnl_{qc}_{h}")
                nc.scalar.activation(lnl, ls, mybir.ActivationFunctionType.Ln)
                rec = vecp.tile([128, 512], F32, tag="rec", bufs=3,
                                name=f"rec_{qc}_{h}")
                nc.scalar.activation(rec, lnl, mybir.ActivationFunctionType.Exp,
                                     scale=-1.0)
                otn = otnp.tile([128, 512], BF16, tag="otn", bufs=8,
                                name=f"otn_{qc}_{h}")
                nc.vector.tensor_mul(otn, ot, rec)
                otns[(qc, h)] = otn
                # interleave previous chunk's o_proj into this head loop
                if qc >= 1:
                    emit_oproj(qc - 1, h)
            if qc == 3:
                for tt in range(4):
                    emit_oproj(3, tt)
    return nc


def _masks():
    # paired layout: [k, dp, half*512 + q] covers diagonal j-tiles
    # (4qc+2dp, 4qc+2dp+1) of a 512-wide q chunk
    kl = np.arange(128)[:, None, None]
    dp = np.arange(2)[None, :, None]
    qq = np.arange(1024)[None, None, :]
    half = qq // 512
    ql = qq % 512
    return (dp * 256 + half * 128 + kl <= ql).astype(ml_dtypes.bfloat16)


def kernel(x, w_q, w_kv, w_o):
    global LAST_RESULTS
    if "nc" not in _CACHE:
        _CACHE["nc"] = _build_program()
        _CACHE["msk"] = _masks()
    nc = _CACHE["nc"]
    bf = ml_dtypes.bfloat16
    x = np.asarray(x, dtype=np.float32)
    w_q = np.asarray(w_q, dtype=np.float32)
    w_kv = np.asarray(w_kv, dtype=np.float32)
    w_o = np.asarray(w_o, dtype=np.float32)

    in_maps = []
    for c in range(NCORES):
        b, g = c // 4, c % 4
        # x[b]: [T, H] -> [p, t4, c, t]
        xbh = np.ascontiguousarray(
            x[b].reshape(4, 512, HC, 128).transpose(3, 0, 2, 1)).astype(bf)
        # w_q rows for this core's 4 heads: [512, H] -> [p, c, m]
        wqg = np.ascontiguousarray(
            w_q[512 * g:512 * (g + 1), :].T.reshape(HC, 128, 512)
            .transpose(1, 0, 2)).astype(bf)
        wkg = np.ascontiguousarray(
            w_kv[128 * g:128 * (g + 1), :].T.reshape(HC, 128, 128)
            .transpose(1, 0, 2)).astype(bf)
        wvg = np.ascontiguousarray(
            w_kv[512 + 128 * g:512 + 128 * (g + 1), :].T.reshape(HC, 128, 128)
            .transpose(1, 0, 2)).astype(bf)
        # w_o cols for this core's heads: [H, 512] -> [p(d), h, e]
        wog = np.ascontiguousarray(
            w_o[:, 512 * g:512 * (g + 1)].T.reshape(G, 128, HIDDEN)
            .transpose(1, 0, 2)).astype(bf)
        in_maps.append({
            "xb": xbh, "wq": wqg, "wk": wkg, "wv": wvg, "wo": wog,
            "msk": _CACHE["msk"],
        })

    res = run_bass_kernel_spmd(nc, in_maps, core_ids=list(range(NCORES)))
    LAST_RESULTS = res
    outs = res.results
    o = [outs[c]["out"].astype(np.float32) for c in range(NCORES)]
    out = np.stack([o[0] + o[1] + o[2] + o[3], o[4] + o[5] + o[6] + o[7]])
    return out


# revision 22
# speedup vs baseline: 1.0635x; 1.0635x over previous
"""Grouped-Query Attention (B=2, T=2048, H=2048, 16 q-heads, 4 kv-heads, d=128,
causal) on 8 Trainium2 NeuronCores.

Sharding: core c = (batch b, kv-group g) with b = c // 4, g = c % 4.
Each core handles one batch element, one kv head, and its 4 q heads:
  - Q/K/V projections for its slice (tensor-parallel over heads)
  - causal attention for 4 q heads against the shared K/V head
  - partial o_proj (row-parallel): out_partial = O_heads @ w_o[:, cols].T
Host sums the 4 per-batch partials (the row-parallel all-reduce) and stacks.

Device layouts (chosen so no transposes are ever needed on-chip):
  QT, KT: [d=128, T]  (projection computed directly transposed)
  V:      [T-tile=128, d]
  scores: computed directly transposed as ST [k, q] via lhsT=KT_j, rhs=QT
  P = exp(ST/sqrt(d)) stays [k, q] and feeds PV as rhs -> OT [d, q] which is
  exactly the lhsT the o_proj needs. Row sums of P (softmax denominator) are
  computed broadcast via an all-ones [128,128] stationary matmul.
All matmul inputs bf16, PSUM accumulation fp32, softmax in fp32.

Perf structure:
  - every input tensor is repacked on the host so its DMA is a single
    transfer (or a few) with long contiguous per-partition rows; the DMA
    queue costs ~625ns per transfer to issue, so many small transfers
    starve the PE.
  - PE warm-up matmuls at t=0 release the HAM clock gate early.
  - score tiles st are single-bank [128,512] PSUM tiles with bufs=4 so the
    PE can run several tiles ahead of the scalar-engine exp.
  - o_proj for chunk qc is emitted interleaved into the attention head loop
    of chunk qc+1 (software pipelining) so PSUM->SBUF casts and stage DMAs
    never serialize against the PE.
  - softmax: denominator reciprocal via DVE reciprocal_approx_fast (~51 ULP,
    5x faster than exact); the OT*(1/l) normalize runs on gpsimd, as does one
    of the four diagonal mask multiplies, to keep DVE under the PE's rate.
  - o_proj PSUM->bf16 casts alternate between scalar and vector engines.
"""

import numpy as np
import ml_dtypes
from contextlib import ExitStack

import concourse.bass as bass
import concourse.mybir as mybir
import concourse.tile as tile
from concourse.bass_utils import run_bass_kernel_spmd

# ---------------------------------------------------------------------------
# Workaround for this compiler build's per-instruction sync-wait-slot limit
# (walrus setupSyncWait rejects >2 waits on an instruction). Post-process the
# serialized BIR: any instruction carrying more than 2 sem waits gets the
# excess moved onto injected same-engine Drain instructions placed directly
# before it (same queue, program order => identical semantics).
import json as _json

_WAIT_LIMITS = {}
_WAIT_LIMIT_DEFAULT = 1
_orig_to_json_bytes = bass.Bass.to_json_bytes


def _split_waits_json(bj: bytes) -> bytes:
    m = _json.loads(bj)
    ctr = 0
    changed = False
    for f in m["functions"]:
        for blk in f["blocks"]:
            out = []
            for inst in blk["instructions"]:
                si = inst.get("sync_info") or {}
                w = si.get("on_wait") or []
                lim = _WAIT_LIMITS.get(inst.get("opcode"), _WAIT_LIMIT_DEFAULT)
                if len(w) > lim:
                    changed = True
                    extra, keep = w[:-lim], w[-lim:]
                    si["on_wait"] = keep
                    for i in range(0, len(extra), 1):
                        ctr += 1
                        out.append({
                            "debug": inst.get("debug", 0),
                            "engine": inst["engine"],
                            "ins": [],
                            "is_reset_sema": False,
                            "name": f"I-wsplit-{ctr}",
                            "opcode": "Drain",
                            "outs": [],
                            "sync_info": {
                                "on_update": [],
                                "on_wait": extra[i:i + 1],
                            },
                        })
                out.append(inst)
            if changed:
                blk["instructions"] = out
    if not changed:
        return bj
    return _json.dumps(m).encode()


def _to_json_bytes_patched(self, *a, **k):
    return _split_waits_json(_orig_to_json_bytes(self, *a, **k))


bass.Bass.to_json_bytes = _to_json_bytes_patched
# ---------------------------------------------------------------------------

HIDDEN = 2048
N_HEADS = 16
N_KV = 4
HD = 128
B, T = 2, 2048
G = N_HEADS // N_KV          # q heads per core = 4
HC = HIDDEN // 128           # contraction chunks = 16
NCORES = 8
SCALE = HD ** -0.5

BF16 = mybir.dt.bfloat16
F32 = mybir.dt.float32

_CACHE = {}
LAST_RESULTS = None


def _build_program():
    nc = bass.Bass("TRN2")
    # host-repacked inputs: partition dim first, long contiguous rows
    xb = nc.dram_tensor("xb", [128, 4, HC, 512], BF16, kind="ExternalInput")
    wq = nc.dram_tensor("wq", [128, HC, G * HD], BF16, kind="ExternalInput")
    wk = nc.dram_tensor("wk", [128, HC, HD], BF16, kind="ExternalInput")
    wv = nc.dram_tensor("wv", [128, HC, HD], BF16, kind="ExternalInput")
    wo = nc.dram_tensor("wo", [128, G, HIDDEN], BF16, kind="ExternalInput")
    msk = nc.dram_tensor("msk", [128, G, 512], BF16, kind="ExternalInput")
    out = nc.dram_tensor("out", [T, HIDDEN], BF16, kind="ExternalOutput")

    EXP = mybir.ActivationFunctionType.Exp

    with tile.TileContext(nc) as tc, ExitStack() as ctx:
        sing = ctx.enter_context(tc.tile_pool(name="sing", bufs=1))
        ptp = ctx.enter_context(tc.tile_pool(name="ptp", bufs=16))
        vecp = ctx.enter_context(tc.tile_pool(name="vecp", bufs=3))
        otnp = ctx.enter_context(tc.tile_pool(name="otnp", bufs=8))
        outp = ctx.enter_context(tc.tile_pool(name="outp", bufs=3))
        psum = ctx.enter_context(tc.tile_pool(name="psum", bufs=2, space="PSUM"))

        xT_sb = sing.tile([128, 4, HC, 512], BF16)
        wq_sb = sing.tile([128, HC, G * HD], BF16)
        wk_sb = sing.tile([128, HC, HD], BF16)
        wv_sb = sing.tile([128, HC, HD], BF16)
        wo_sb = sing.tile([128, G, HIDDEN], BF16)
        msk_sb = sing.tile([128, G, 512], BF16)
        ones_sb = sing.tile([128, 128], BF16)
        warm_sb = sing.tile([128, 512], BF16)
        qt_sb = sing.tile([128, G, T], BF16)
        kt_sb = sing.tile([128, T], BF16)
        vt_sb = sing.tile([128, T], BF16)
        v_sb = sing.tile([128, HC, HD], BF16)

        nc.vector.memset(ones_sb, 1.0)
        nc.vector.memset(warm_sb, 0.0)

        # --- PE warm-up: release the HAM clock gate before real work ---
        for w in range(16):
            wp = psum.tile([128, 512], F32, tag="st", bufs=4, name=f"warm_{w}")
            nc.tensor.matmul(wp, lhsT=ones_sb, rhs=warm_sb, start=True, stop=True)

        # --- input DMAs: few big transfers, K/V/x-block0 first ---
        nc.sync.dma_start(out=wk_sb, in_=wk[:, :, :])
        for qtr in range(4):
            nc.sync.dma_start(out=xT_sb[:, 0, 4 * qtr:4 * qtr + 4],
                              in_=xb[:, 0, 4 * qtr:4 * qtr + 4])
        nc.sync.dma_start(out=wv_sb, in_=wv[:, :, :])
        nc.sync.dma_start(out=xT_sb[:, 1], in_=xb[:, 1])
        nc.sync.dma_start(out=wq_sb, in_=wq[:, :, :])
        for t4 in range(2, 4):
            nc.sync.dma_start(out=xT_sb[:, t4], in_=xb[:, t4])
        nc.sync.dma_start(out=msk_sb, in_=msk[:, :, :])
        nc.sync.dma_start(out=wo_sb, in_=wo[:, :, :])

        # ---- projections (per T-chunk) ----
        for t4 in range(4):
            tsl = slice(t4 * 512, (t4 + 1) * 512)
            # K projection -> KT [d, 512]
            kp = psum.tile([128, 512], F32, tag="ls", bufs=2, name=f"kp_{t4}")
            for c in range(HC):
                nc.tensor.matmul(
                    kp, lhsT=wk_sb[:, c, :], rhs=xT_sb[:, t4, c, :],
                    start=(c == 0), stop=(c == HC - 1),
                )
            nc.scalar.copy(kt_sb[:, tsl], kp)
            # V projection -> VT [d, 512], then XBAR-transpose to V [t, d]
            vtp = psum.tile([128, 512], F32, tag="ot", bufs=2, name=f"vtp_{t4}")
            for c in range(HC):
                nc.tensor.matmul(
                    vtp, lhsT=wv_sb[:, c, :], rhs=xT_sb[:, t4, c, :],
                    start=(c == 0), stop=(c == HC - 1),
                )
            nc.scalar.copy(vt_sb[:, tsl], vtp)
            for ts in range(4):
                tt = 4 * t4 + ts
                nc.sync.dma_start_transpose(
                    out=v_sb[:, tt, :], in_=vt_sb[:, tt * 128:(tt + 1) * 128])
            # Q projection -> QT [d, h, 512]
            for h in range(G):
                qp = psum.tile([128, 512], F32, tag="st", bufs=4,
                               name=f"qp_{t4}_{h}")
                for c in range(HC):
                    nc.tensor.matmul(
                        qp, lhsT=wq_sb[:, c, h * HD:(h + 1) * HD],
                        rhs=xT_sb[:, t4, c, :],
                        start=(c == 0), stop=(c == HC - 1),
                    )
                nc.scalar.copy(qt_sb[:, h, tsl], qp)

        # ---- attention with o_proj software-pipelined one chunk behind ----
        otns = {}

        def emit_oproj(qc, tt):
            stage = outp.tile([128, HIDDEN], BF16, tag="stage", bufs=3,
                              name=f"stage_{qc}_{tt}")
            for ec in range(4):
                op = psum.tile([128, 512], F32, tag="st", bufs=4,
                               name=f"op_{qc}_{tt}_{ec}")
                for h in range(G):
                    nc.tensor.matmul(
                        op, lhsT=otns[(qc, h)][:, tt * 128:(tt + 1) * 128],
                        rhs=wo_sb[:, h, ec * 512:(ec + 1) * 512],
                        start=(h == 0), stop=(h == G - 1),
                    )
                esl = slice(ec * 512, (ec + 1) * 512)
                nc.vector.tensor_copy(stage[:, esl], op)
            r0 = qc * 512 + tt * 128
            nc.sync.dma_start(out=out[r0:r0 + 128, :], in_=stage)

        for qc in range(4):
            qsl = slice(qc * 512, (qc + 1) * 512)
            njt = 4 * qc + 4
            # diagonal j-tiles first so their mask ops don't pile up late
            js = list(range(4 * qc, njt)) + list(range(0, 4 * qc))
            for h in range(G):
                ot = psum.tile([128, 512], F32, tag="ot", bufs=2,
                               name=f"ot_{qc}_{h}")
                ls = psum.tile([128, 512], F32, tag="ls", bufs=2,
                               name=f"ls_{qc}_{h}")
                # software pipeline: PV/LS lag the S matmuls by several
                # tiles so the exp (+ mask) latency never stalls the
                # in-order PE queue.
                depth = min(njt, 6)
                pend = []
                n_emitted = 0

                def flush_one():
                    nonlocal n_emitted
                    jj, pp = pend.pop(0)
                    nc.tensor.matmul(ot, lhsT=v_sb[:, jj, :], rhs=pp,
                                     start=(n_emitted == 0),
                                     stop=(n_emitted == njt - 1))
                    nc.tensor.matmul(ls, lhsT=ones_sb, rhs=pp,
                                     start=(n_emitted == 0),
                                     stop=(n_emitted == njt - 1))
                    n_emitted += 1

                for j in js:
                    st = psum.tile([128, 512], F32, tag="st", bufs=4,
                                   name=f"st_{qc}_{h}_{j}")
                    nc.tensor.matmul(
                        st, lhsT=kt_sb[:, j * 128:(j + 1) * 128],
                        rhs=qt_sb[:, h, qsl], start=True, stop=True,
                    )
                    pt = ptp.tile([128, 512], BF16, tag="pt", bufs=16,
                                  name=f"pt_{qc}_{h}_{j}")
                    nc.scalar.activation(pt, st, EXP, scale=float(SCALE))
                    if j >= 4 * qc:
                        nc.vector.tensor_mul(pt, pt, msk_sb[:, j - 4 * qc, :])
                    pend.append((j, pt))
                    if len(pend) > depth:
                        flush_one()
                while pend:
                    flush_one()
                lnl = vecp.tile([128, 512], F32, tag="lnl", bufs=3,
                                name=f"lnl_{qc}_{h}")
                nc.scalar.activation(lnl, ls, mybir.ActivationFunctionType.Ln)
                rec = vecp.tile([128, 512], F32, tag="rec", bufs=3,
                                name=f"rec_{qc}_{h}")
                nc.scalar.activation(rec, lnl, mybir.ActivationFunctionType.Exp,
                                     scale=-1.0)
                otn = otnp.tile([128, 512], BF16, tag="otn", bufs=8,
                                name=f"otn_{qc}_{h}")
                nc.vector.tensor_mul(otn, ot, rec)
                otns[(qc, h)] = otn
                # interleave previous chunk's o_proj into this head loop
                if qc >= 1:
                    emit_oproj(qc - 1, h)
            if qc == 3:
                for tt in range(4):
                    emit_oproj(3, tt)
    return nc


def _masks():
    kl = np.arange(128)[:, None, None]
    jj = np.arange(G)[None, :, None]
    ql = np.arange(512)[None, None, :]
    return (128 * jj + kl <= ql).astype(ml_dtypes.bfloat16)


def kernel(x, w_q, w_kv, w_o):
    global LAST_RESULTS
    if "nc" not in _CACHE:
        _CACHE["nc"] = _build_program()
        _CACHE["msk"] = _masks()
    nc = _CACHE["nc"]
    bf = ml_dtypes.bfloat16
    x = np.asarray(x, dtype=np.float32)
    w_q = np.asarray(w_q, dtype=np.float32)
    w_kv = np.asarray(w_kv, dtype=np.float32)
    w_o = np.asarray(w_o, dtype=np.float32)

    in_maps = []
    for c in range(NCORES):
        b, g = c // 4, c % 4
        # x[b]: [T, H] -> [p, t4, c, t]
        xbh = np.ascontiguousarray(
            x[b].reshape(4, 512, HC, 128).transpose(3, 0, 2, 1)).astype(bf)
        # w_q rows for this core's 4 heads: [512, H] -> [p, c, m]
        wqg = np.ascontiguousarray(
            w_q[512 * g:512 * (g + 1), :].T.reshape(HC, 128, 512)
            .transpose(1, 0, 2)).astype(bf)
        wkg = np.ascontiguousarray(
            w_kv[128 * g:128 * (g + 1), :].T.reshape(HC, 128, 128)
            .transpose(1, 0, 2)).astype(bf)
        wvg = np.ascontiguousarray(
            w_kv[512 + 128 * g:512 + 128 * (g + 1), :].T.reshape(HC, 128, 128)
            .transpose(1, 0, 2)).astype(bf)
        # w_o cols for this core's heads: [H, 512] -> [p(d), h, e]
        wog = np.ascontiguousarray(
            w_o[:, 512 * g:512 * (g + 1)].T.reshape(G, 128, HIDDEN)
            .transpose(1, 0, 2)).astype(bf)
        in_maps.append({
            "xb": xbh, "wq": wqg, "wk": wkg, "wv": wvg, "wo": wog,
            "msk": _CACHE["msk"],
        })

    res = run_bass_kernel_spmd(nc, in_maps, core_ids=list(range(NCORES)))
    LAST_RESULTS = res
    outs = res.results
    o = [outs[c]["out"].astype(np.float32) for c in range(NCORES)]
    out = np.stack([o[0] + o[1] + o[2] + o[3], o[4] + o[5] + o[6] + o[7]])
    return out
